# revision 2
# baseline (speedup 1.0000x reference)
"""Trainium2 Bass kernel for Luong dot attention + softmax.

  queries: [1, 64, 1024] f32
  keys:    [4096, 64, 1024] f32
  out:     [1, 64, 4096] f32 = softmax_s(einsum('bh,sbh->bs', q[0], keys))

Sharding: data-parallel over batch. Core m handles batches [8m, 8m+8).
Per core this is memory-bound: 128 MiB of keys streamed from HBM.

Per-core kernel:
  - keys shard viewed as [4096, 8*1024]; 32 s-tiles of [128, 8192]
    (32 KiB contiguous per partition per DMA -> near-peak HBM bandwidth).
  - q replicated host-side to [128, 8192] so the DVE can read it per-lane.
  - One fused tensor_tensor_reduce per (s-tile, batch): multiplies
    K-tile slice by q-replica and row-reduces to the [128,1] score column.
  - Softmax over s=4096 (partitions x 32 columns): free-dim reduce_max,
    gpsimd partition_all_reduce for the cross-partition max/sum, ScalarE
    Exp with per-partition bias and fused accum, reciprocal + scale.
  - Scores live as S[p, b, t] with s = t*128 + p; host untransposes.
"""

import numpy as np

N_CORES = 8
SEQ = 4096
B_TOT = 64
H = 1024
P = 128
B = B_TOT // N_CORES          # 8 batches per core
NT = SEQ // P                 # 32 s-tiles
FB = B * H                    # 8192 free elems per s-tile

_PROGRAM = None


def _build_program():
    """Build + compile the Bass/Tile program once per process."""
    import concourse.bass as bass  # noqa: F401
    import concourse.mybir as mybir
    import concourse.bass_isa as bass_isa
    from concourse import bacc, tile

    f32 = mybir.dt.float32
    nc = bacc.Bacc("TRN2", target_bir_lowering=False, debug=False,
                   num_devices=N_CORES)

    keys_d = nc.dram_tensor("keys", [SEQ, FB], f32, kind="ExternalInput")
    qrep_d = nc.dram_tensor("qrep", [P, FB], f32, kind="ExternalInput")
    out_d = nc.dram_tensor("out", [P, B, NT], f32, kind="ExternalOutput")

    with tile.TileContext(nc) as tc:
        with (
            tc.tile_pool(name="kpool", bufs=3) as kpool,
            tc.tile_pool(name="qpool", bufs=1) as qpool,
            tc.tile_pool(name="ppool", bufs=4) as ppool,
            tc.tile_pool(name="spool", bufs=1) as spool,
        ):
            qrep = qpool.tile([P, FB], f32)
            nc.sync.dma_start(qrep[:, :], qrep_d.ap()[:, :])

            s_all = spool.tile([P, B, NT], f32)   # raw scores
            e_all = spool.tile([P, B, NT], f32)   # exp(scores - max)
            o_all = spool.tile([P, B, NT], f32)   # softmax output
            m1 = spool.tile([P, B], f32)          # per-partition max
            gm = spool.tile([P, B], f32)          # global max (all parts)
            negm = spool.tile([P, B], f32)        # -global max
            r1 = spool.tile([P, B], f32)          # per-partition exp sum
            gr = spool.tile([P, B], f32)          # global exp sum
            rr = spool.tile([P, B], f32)          # 1 / global sum

            for t in range(NT):
                ktile = kpool.tile([P, FB], f32, tag="ktile")
                nc.sync.dma_start(ktile[:, :], keys_d.ap()[t * P:(t + 1) * P, :])
                for b in range(B):
                    # tensor_tensor_reduce would fuse these, but it crashes
                    # the NRT exec unit on this runtime; split across DVE
                    # (multiply) + ScalarE (in-place Copy with row-sum accum)
                    # so the reduce costs no DVE time.
                    pscr = ppool.tile([P, H], f32, tag="pscr")
                    nc.vector.tensor_mul(
                        pscr[:, :],
                        ktile[:, b * H:(b + 1) * H],
                        qrep[:, b * H:(b + 1) * H],
                    )
                    nc.scalar.activation(
                        pscr[:, :], pscr[:, :],
                        mybir.ActivationFunctionType.Copy,
                        accum_out=s_all[:, b, t:t + 1],
                    )

            # ---- softmax over s (partitions x NT columns), per batch ----
            nc.vector.reduce_max(m1[:, :], s_all[:, :, :],
                                 axis=mybir.AxisListType.X)
            nc.gpsimd.partition_all_reduce(gm[:, :], m1[:, :], channels=P,
                                           reduce_op=bass_isa.ReduceOp.max)
            nc.vector.tensor_scalar_mul(negm[:, :], gm[:, :], -1.0)
            for b in range(B):
                nc.scalar.activation(
                    e_all[:, b, :], s_all[:, b, :],
                    mybir.ActivationFunctionType.Exp,
                    bias=negm[:, b:b + 1], scale=1.0,
                    accum_out=r1[:, b:b + 1],
                )
            nc.gpsimd.partition_all_reduce(gr[:, :], r1[:, :], channels=P,
                                           reduce_op=bass_isa.ReduceOp.add)
            nc.vector.reciprocal(rr[:, :], gr[:, :])
            for b in range(B):
                nc.vector.tensor_scalar_mul(o_all[:, b, :], e_all[:, b, :],
                                            rr[:, b:b + 1])
            nc.sync.dma_start(out_d.ap()[:, :, :], o_all[:, :, :])

    nc.compile()
    return nc


def _get_program():
    global _PROGRAM
    if _PROGRAM is None:
        _PROGRAM = _build_program()
    return _PROGRAM


def _make_in_maps(queries, keys):
    queries = np.ascontiguousarray(queries, dtype=np.float32)
    keys = np.ascontiguousarray(keys, dtype=np.float32)
    in_maps = []
    for m in range(N_CORES):
        lo, hi = m * B, (m + 1) * B
        ks = np.ascontiguousarray(keys[:, lo:hi, :]).reshape(SEQ, FB)
        q = queries[0, lo:hi, :].reshape(FB)
        qrep = np.ascontiguousarray(np.broadcast_to(q, (P, FB)))
        in_maps.append({"keys": ks, "qrep": qrep})
    return in_maps


def _run(queries, keys, **spmd_kwargs):
    from concourse import bass_utils

    nc = _get_program()
    in_maps = _make_in_maps(queries, keys)
    res = bass_utils.run_bass_kernel_spmd(
        nc, in_maps, core_ids=list(range(N_CORES)), **spmd_kwargs
    )
    outs = []
    for m in range(N_CORES):
        o = np.asarray(res.results[m]["out"]).reshape(P, B, NT)
        # o[p, b, t] = score(batch m*B+b, s = t*128 + p)
        outs.append(o.transpose(1, 2, 0).reshape(B, SEQ))
    full = np.concatenate(outs, axis=0)[None]  # [1, 64, 4096]
    return np.ascontiguousarray(full.astype(np.float32)), res


def kernel(queries, keys):
    out, _ = _run(queries, keys)
    return out


# revision 5
# speedup vs baseline: 20.2969x; 20.2969x over previous
"""Trainium2 Bass kernel for Luong dot attention + softmax.

  queries: [1, 64, 1024] f32
  keys:    [4096, 64, 1024] f32
  out:     [1, 64, 4096] f32 = softmax_s(einsum('bh,sbh->bs', q[0], keys))

Sharding: data-parallel over batch. Core m handles batches [8m, 8m+8).
Per core this is memory-bound: 128 MiB of keys streamed from HBM.

Per-core kernel:
  - keys shard viewed as [4096, 8*1024]; 32 s-tiles of [128, 8192]
    (32 KiB contiguous per partition per DMA -> near-peak HBM bandwidth).
  - q replicated host-side to [128, 8192] so the DVE can read it per-lane.
  - One fused tensor_tensor_reduce per (s-tile, batch): multiplies
    K-tile slice by q-replica and row-reduces to the [128,1] score column.
  - Softmax over s=4096 (partitions x 32 columns): free-dim reduce_max,
    gpsimd partition_all_reduce for the cross-partition max/sum, ScalarE
    Exp with per-partition bias and fused accum, reciprocal + scale.
  - Scores live as S[p, b, t] with s = t*128 + p; host untransposes.
"""

import numpy as np

N_CORES = 8
SEQ = 4096
B_TOT = 64
H = 1024
P = 128
B = B_TOT // N_CORES          # 8 batches per core
NT = SEQ // P                 # 32 s-tiles
FB = B * H                    # 8192 free elems per s-tile
CH = 4                        # batches per DVE multiply instruction

_PROGRAM = None


def _build_program():
    """Build + compile the Bass/Tile program once per process."""
    import concourse.bass as bass  # noqa: F401
    import concourse.mybir as mybir
    import concourse.bass_isa as bass_isa
    from concourse import bacc, tile

    f32 = mybir.dt.float32
    nc = bacc.Bacc("TRN2", target_bir_lowering=False, debug=False,
                   num_devices=N_CORES)

    keys_d = nc.dram_tensor("keys", [SEQ, FB], f32, kind="ExternalInput")
    qrep_d = nc.dram_tensor("qrep", [P, FB], f32, kind="ExternalInput")
    out_d = nc.dram_tensor("out", [P, B, NT], f32, kind="ExternalOutput")

    with tile.TileContext(nc) as tc:
        with (
            tc.tile_pool(name="kpool", bufs=3) as kpool,
            tc.tile_pool(name="qpool", bufs=1) as qpool,
            tc.tile_pool(name="ppool", bufs=3) as ppool,
            tc.tile_pool(name="spool", bufs=1) as spool,
        ):
            qrep = qpool.tile([P, FB], f32)
            nc.sync.dma_start(qrep[:, :], qrep_d.ap()[:, :])

            s_all = spool.tile([P, B, NT], f32)   # raw scores
            e_all = spool.tile([P, B, NT], f32)   # exp(scores - max)
            o_all = spool.tile([P, B, NT], f32)   # softmax output
            m1 = spool.tile([P, B], f32)          # per-partition max
            gm = spool.tile([P, B], f32)          # global max (all parts)
            negm = spool.tile([P, B], f32)        # -global max
            r1 = spool.tile([P, B], f32)          # per-partition exp sum
            gr = spool.tile([P, B], f32)          # global exp sum
            rr = spool.tile([P, B], f32)          # 1 / global sum

            for t in range(NT):
                ktile = kpool.tile([P, FB], f32, tag="ktile")
                nc.sync.dma_start(ktile[:, :], keys_d.ap()[t * P:(t + 1) * P, :])
                # tensor_tensor_reduce would fuse multiply+reduce in one DVE
                # op, but it crashes the NRT exec unit on this runtime; split
                # across DVE (one wide multiply per CH batches) + ScalarE
                # (in-place Copy with row-sum accum per batch) so the reduce
                # costs no DVE time.
                for b0 in range(0, B, CH):
                    pscr = ppool.tile([P, CH * H], f32, tag="pscr")
                    nc.vector.tensor_mul(
                        pscr[:, :],
                        ktile[:, b0 * H:(b0 + CH) * H],
                        qrep[:, b0 * H:(b0 + CH) * H],
                    )
                    for j in range(CH):
                        nc.scalar.activation(
                            pscr[:, j * H:(j + 1) * H],
                            pscr[:, j * H:(j + 1) * H],
                            mybir.ActivationFunctionType.Copy,
                            accum_out=s_all[:, b0 + j, t:t + 1],
                        )

            # ---- softmax over s (partitions x NT columns), per batch ----
            nc.vector.reduce_max(m1[:, :], s_all[:, :, :],
                                 axis=mybir.AxisListType.X)
            nc.gpsimd.partition_all_reduce(gm[:, :], m1[:, :], channels=P,
                                           reduce_op=bass_isa.ReduceOp.max)
            nc.vector.tensor_scalar_mul(negm[:, :], gm[:, :], -1.0)
            for b in range(B):
                nc.scalar.activation(
                    e_all[:, b, :], s_all[:, b, :],
                    mybir.ActivationFunctionType.Exp,
                    bias=negm[:, b:b + 1], scale=1.0,
                    accum_out=r1[:, b:b + 1],
                )
            nc.gpsimd.partition_all_reduce(gr[:, :], r1[:, :], channels=P,
                                           reduce_op=bass_isa.ReduceOp.add)
            nc.vector.reciprocal(rr[:, :], gr[:, :])
            for b in range(B):
                nc.vector.tensor_scalar_mul(o_all[:, b, :], e_all[:, b, :],
                                            rr[:, b:b + 1])
            nc.sync.dma_start(out_d.ap()[:, :, :], o_all[:, :, :])

    nc.compile()
    return nc


def _get_program():
    global _PROGRAM
    if _PROGRAM is None:
        _PROGRAM = _build_program()
    return _PROGRAM


def _make_in_maps(queries, keys):
    queries = np.ascontiguousarray(queries, dtype=np.float32)
    keys = np.ascontiguousarray(keys, dtype=np.float32)
    in_maps = []
    for m in range(N_CORES):
        lo, hi = m * B, (m + 1) * B
        ks = np.ascontiguousarray(keys[:, lo:hi, :]).reshape(SEQ, FB)
        q = queries[0, lo:hi, :].reshape(FB)
        qrep = np.ascontiguousarray(np.broadcast_to(q, (P, FB)))
        in_maps.append({"keys": ks, "qrep": qrep})
    return in_maps


def _run(queries, keys, **spmd_kwargs):
    from concourse import bass_utils

    nc = _get_program()
    in_maps = _make_in_maps(queries, keys)
    res = bass_utils.run_bass_kernel_spmd(
        nc, in_maps, core_ids=list(range(N_CORES)), **spmd_kwargs
    )
    outs = []
    for m in range(N_CORES):
        o = np.asarray(res.results[m]["out"]).reshape(P, B, NT)
        # o[p, b, t] = score(batch m*B+b, s = t*128 + p)
        outs.append(o.transpose(1, 2, 0).reshape(B, SEQ))
    full = np.concatenate(outs, axis=0)[None]  # [1, 64, 4096]
    return np.ascontiguousarray(full.astype(np.float32)), res


def kernel(queries, keys):
    out, _ = _run(queries, keys)
    return out


# revision 7
# speedup vs baseline: 23.0198x; 1.1342x over previous
"""Trainium2 Bass kernel for Luong dot attention + softmax.

  queries: [1, 64, 1024] f32
  keys:    [4096, 64, 1024] f32
  out:     [1, 64, 4096] f32 = softmax_s(einsum('bh,sbh->bs', q[0], keys))

Sharding: data-parallel over batch. Core m handles batches [8m, 8m+8).
Per core this is memory-bound: 128 MiB of keys streamed from HBM.

Per-core kernel:
  - keys shard viewed as [4096, 8*1024]; 32 s-tiles of [128, 8192]
    (32 KiB contiguous per partition per DMA -> near-peak HBM bandwidth).
  - q replicated host-side to [128, 8192] so the DVE can read it per-lane.
  - One fused tensor_tensor_reduce per (s-tile, batch): multiplies
    K-tile slice by q-replica and row-reduces to the [128,1] score column.
  - Softmax over s=4096 (partitions x 32 columns): free-dim reduce_max,
    gpsimd partition_all_reduce for the cross-partition max/sum, ScalarE
    Exp with per-partition bias and fused accum, reciprocal + scale.
  - Scores live as S[p, b, t] with s = t*128 + p; host untransposes.
"""

import numpy as np

N_CORES = 8
SEQ = 4096
B_TOT = 64
H = 1024
P = 128
B = B_TOT // N_CORES          # 8 batches per core
NT = SEQ // P                 # 32 s-tiles
FB = B * H                    # 8192 free elems per s-tile
CH = 4                        # batches per DVE multiply instruction

_PROGRAM = None


def _build_program():
    """Build + compile the Bass/Tile program once per process."""
    import concourse.bass as bass  # noqa: F401
    import concourse.mybir as mybir
    import concourse.bass_isa as bass_isa
    from concourse import bacc, tile

    f32 = mybir.dt.float32
    nc = bacc.Bacc("TRN2", target_bir_lowering=False, debug=False,
                   num_devices=N_CORES)

    keys_d = nc.dram_tensor("keys", [SEQ, FB], f32, kind="ExternalInput")
    qrep_d = nc.dram_tensor("qrep", [P, FB], f32, kind="ExternalInput")
    out_d = nc.dram_tensor("out", [P, B, NT], f32, kind="ExternalOutput")

    with tile.TileContext(nc) as tc:
        with (
            tc.tile_pool(name="kpool", bufs=6) as kpool,
            tc.tile_pool(name="qpool", bufs=1) as qpool,
            tc.tile_pool(name="ppool", bufs=3) as ppool,
            tc.tile_pool(name="spool", bufs=1) as spool,
        ):
            qrep = qpool.tile([P, FB], f32)
            nc.sync.dma_start(qrep[:, :], qrep_d.ap()[:, :])

            s_all = spool.tile([P, B, NT], f32)   # raw scores
            e_all = spool.tile([P, B, NT], f32)   # exp(scores - max)
            o_all = spool.tile([P, B, NT], f32)   # softmax output
            m1 = spool.tile([P, B], f32)          # per-partition max
            gm = spool.tile([P, B], f32)          # global max (all parts)
            negm = spool.tile([P, B], f32)        # -global max
            r1 = spool.tile([P, B], f32)          # per-partition exp sum
            gr = spool.tile([P, B], f32)          # global exp sum
            rr = spool.tile([P, B], f32)          # 1 / global sum

            # tensor_tensor_reduce would fuse multiply+reduce in one DVE op,
            # but it crashes the NRT exec unit on this runtime; split across
            # DVE (one wide multiply per CH batches) + ScalarE (in-place Copy
            # with row-sum accum per batch) so the reduce costs no DVE time.
            # 2 MiB half-tile DMAs (CH batches each) pipeline more finely
            # than one 4 MiB DMA per s-tile.
            HF = CH * H
            for t in range(NT):
                for h0 in range(B // CH):
                    ktile = kpool.tile([P, HF], f32, tag="ktile")
                    nc.sync.dma_start(
                        ktile[:, :],
                        keys_d.ap()[t * P:(t + 1) * P, h0 * HF:(h0 + 1) * HF])
                    pscr = ppool.tile([P, HF], f32, tag="pscr")
                    nc.vector.tensor_mul(pscr[:, :], ktile[:, :],
                                         qrep[:, h0 * HF:(h0 + 1) * HF])
                    for j in range(CH):
                        nc.scalar.activation(
                            pscr[:, j * H:(j + 1) * H],
                            pscr[:, j * H:(j + 1) * H],
                            mybir.ActivationFunctionType.Copy,
                            accum_out=s_all[:, h0 * CH + j, t:t + 1],
                        )

            # ---- softmax over s (partitions x NT columns), per batch ----
            nc.vector.reduce_max(m1[:, :], s_all[:, :, :],
                                 axis=mybir.AxisListType.X)
            nc.gpsimd.partition_all_reduce(gm[:, :], m1[:, :], channels=P,
                                           reduce_op=bass_isa.ReduceOp.max)
            nc.vector.tensor_scalar_mul(negm[:, :], gm[:, :], -1.0)
            for b in range(B):
                nc.scalar.activation(
                    e_all[:, b, :], s_all[:, b, :],
                    mybir.ActivationFunctionType.Exp,
                    bias=negm[:, b:b + 1], scale=1.0,
                    accum_out=r1[:, b:b + 1],
                )
            nc.gpsimd.partition_all_reduce(gr[:, :], r1[:, :], channels=P,
                                           reduce_op=bass_isa.ReduceOp.add)
            nc.vector.reciprocal(rr[:, :], gr[:, :])
            for b in range(B):
                nc.vector.tensor_scalar_mul(o_all[:, b, :], e_all[:, b, :],
                                            rr[:, b:b + 1])
            nc.sync.dma_start(out_d.ap()[:, :, :], o_all[:, :, :])

    nc.compile()
    return nc


def _get_program():
    global _PROGRAM
    if _PROGRAM is None:
        _PROGRAM = _build_program()
    return _PROGRAM


def _make_in_maps(queries, keys):
    queries = np.ascontiguousarray(queries, dtype=np.float32)
    keys = np.ascontiguousarray(keys, dtype=np.float32)
    in_maps = []
    for m in range(N_CORES):
        lo, hi = m * B, (m + 1) * B
        ks = np.ascontiguousarray(keys[:, lo:hi, :]).reshape(SEQ, FB)
        q = queries[0, lo:hi, :].reshape(FB)
        qrep = np.ascontiguousarray(np.broadcast_to(q, (P, FB)))
        in_maps.append({"keys": ks, "qrep": qrep})
    return in_maps


def _run(queries, keys, **spmd_kwargs):
    from concourse import bass_utils

    nc = _get_program()
    in_maps = _make_in_maps(queries, keys)
    res = bass_utils.run_bass_kernel_spmd(
        nc, in_maps, core_ids=list(range(N_CORES)), **spmd_kwargs
    )
    outs = []
    for m in range(N_CORES):
        o = np.asarray(res.results[m]["out"]).reshape(P, B, NT)
        # o[p, b, t] = score(batch m*B+b, s = t*128 + p)
        outs.append(o.transpose(1, 2, 0).reshape(B, SEQ))
    full = np.concatenate(outs, axis=0)[None]  # [1, 64, 4096]
    return np.ascontiguousarray(full.astype(np.float32)), res


def kernel(queries, keys):
    out, _ = _run(queries, keys)
    return out
